# revision 18
# baseline (speedup 1.0000x reference)
"""Groupwise asymmetric 4-bit quantize+dequantize (KV-cache RTN) on 8 TRN2 cores.

Reference semantics (per contiguous group of 128 along the last dim):
  scale  = max((max(g) - min(g)) / 15, 1e-8)
  offset = round(-min(g) / scale)
  q      = clip(round(x / scale) + offset, 0, 15)
  out    = (q - offset) * scale
        == min(round(x / scale), hi) * scale,  hi = 15 - offset
  (the lower clamp never fires: round is monotone and x >= min(g))

Key trick: the upper clamp is folded into uint8 OUTPUT SATURATION, verified
on HW to be exactly clip(rint(v), 0, 255) on every engine:

    v  = u8(x*rs + cof),  cof = 255 - hi = 240 + offset
         -> in-range quants land in [240, 255]; u > hi saturates at 255,
            which IS the clamp; the low side (>= 240) never saturates.
    out = fp16(v*sc + hs),  hs = -cof*sc     [ = (v - cof)*sc = w*sc ]

Both passes are then plain (mult, add) tensor_scalar ops with per-partition
[P,1] scalars -- the only op shape that is fast on ALL of DVE (2x mode),
ACT (Identity activation), and Pool (gpsimd software ALU). Work is spread
across those three engines; only the per-group min/max reductions are
DVE-bound (~2.2us per 2048 elems/way, no fast mode exists).

fp16 IO: host converts x f32->fp16, kernel emits fp16, host upcasts
(rel err ~7.7e-3 vs the f32 reference; gate is 2e-2). Halves HBM traffic.
int8/u8 saturation margins: |x*rs| <= 15*|x|/(max-min) stays < 100 for
randn-scale groups; v in [0, 255] by construction.

Sharding: fully elementwise per group -> 8 equal contiguous shards, one per
NeuronCore, no communication. Warm-up/cool-down tiles are quarter-size and
avoid DVE slabs so the reduce pipeline ramps without starving ACT/Pool.
"""

import sys

sys.path.insert(0, "/opt/trn_rl_repo")

import numpy as np

import concourse.bass as bass  # noqa: F401
import concourse.bacc as bacc
import concourse.mybir as mybir
import concourse.tile as tile
from concourse.bass_utils import run_bass_kernel_spmd

# Problem constants (hardcoded per harness contract)
FULL_SHAPE = (4, 32, 4096, 128)
N_CORES = 8
G = 128                      # group size (elements per quant group)
TOTAL = 4 * 32 * 4096 * 128  # 67,108,864 elements
PER_CORE = TOTAL // N_CORES  # 8,388,608 elements
GROUPS_PER_CORE = PER_CORE // G  # 65,536 groups

P = 128                      # SBUF partitions
F = 32                       # groups per partition per tile
TILE_GROUPS = P * F          # 4096 groups per tile
TILE_FREE = F * G            # 4096 elements per partition per tile
N_TILES = GROUPS_PER_CORE // TILE_GROUPS  # 16

M = 12582912.0               # 1.5 * 2**23 (round-to-int magic constant)

# Chain assignment per full tile (tuned on HW traces):
#   f in [0, AP)          ACT P1 -> Pool P2
#   f in [AP, AP+AA)      ACT P1 -> ACT P2
#   f in [AP+AA, AP+AA+PP) Pool P1 -> Pool P2
#   rest                  DVE P1 -> DVE P2  (self-contained, no cross-stall)
AP_N, AA_N, PP_N = 25, 3, 1

_COMPILED = None

AF = mybir.ActivationFunctionType
ALU = mybir.AluOpType
DT = mybir.dt


def _build():
    nc = bacc.Bacc("TRN2", target_bir_lowering=False, debug=False)
    x_d = nc.dram_tensor(
        "x", [GROUPS_PER_CORE, G], DT.float16, kind="ExternalInput"
    ).ap()
    y_d = nc.dram_tensor(
        "y", [GROUPS_PER_CORE, G], DT.float16, kind="ExternalOutput"
    ).ap()

    with tile.TileContext(nc) as tc:
        with (
            tc.tile_pool(name="xp", bufs=6) as xp,
            tc.tile_pool(name="wp", bufs=5) as wp,
            tc.tile_pool(name="op", bufs=5) as op,
            tc.tile_pool(name="st", bufs=6) as st,
        ):
            pending_out = []  # lagged output DMAs so their semaphore waits
            # never block input prefetch at the head of the Sync HWDGE queue

            def flush_out(keep):
                while len(pending_out) > keep:
                    orows, ot = pending_out.pop(0)
                    nc.sync.dma_start(
                        out=orows.rearrange("(p f) g -> p (f g)", p=P), in_=ot[:])

            def emit(row0, nf, ramp=False):
                """One tile of nf groups/partition starting at DRAM row row0."""
                tg = P * nf
                tf = nf * G
                if ramp:  # keep DVE free for the next tile's reduces
                    n_ap = nf - 2
                    n_aa = 1
                    n_pp = 1
                else:
                    n_ap, n_aa, n_pp = AP_N, AA_N, PP_N
                rows = x_d[row0 : row0 + tg, :]
                xh = xp.tile([P, tf], DT.float16, tag="x")
                nc.sync.dma_start(out=xh[:], in_=rows.rearrange("(p f) g -> p (f g)", p=P))
                x3 = xh[:].rearrange("p (f g) -> p f g", g=G)

                mx = st.tile([P, nf], DT.float16, tag="mx")
                mnn = st.tile([P, nf], DT.float16, tag="mnn")  # -min
                nc.vector.tensor_reduce(mx[:], x3, axis=mybir.AxisListType.X, op=ALU.max)
                nc.vector.tensor_reduce(
                    mnn[:], x3, axis=mybir.AxisListType.X, op=ALU.min, negate=True)

                # Per-group constants [P, nf] f32:
                dv = st.tile([P, nf], DT.float32, tag="dv")    # mx - mn
                nc.vector.tensor_tensor(dv[:], mx[:], mnn[:], op=ALU.add)
                sc = st.tile([P, nf], DT.float32, tag="sc")    # scale
                nc.vector.tensor_scalar(
                    sc[:], dv[:], 1.0 / 15.0, 1e-8, op0=ALU.mult, op1=ALU.max)
                scn = st.tile([P, nf], DT.float32, tag="scn")  # -scale
                nc.vector.tensor_scalar(
                    scn[:], dv[:], -1.0 / 15.0, -1e-8, op0=ALU.mult, op1=ALU.min)
                rs = st.tile([P, nf], DT.float32, tag="rs")    # 1/scale
                nc.vector.reciprocal(rs[:], sc[:])
                b2 = st.tile([P, nf], DT.float32, tag="b2")    # -mn/scale
                nc.vector.tensor_tensor(b2[:], mnn[:], rs[:], op=ALU.mult)
                cof = st.tile([P, nf], DT.float32, tag="cof")  # 240 + offset
                nc.vector.tensor_scalar(
                    cof[:], b2[:], M, M - 240.0, op0=ALU.add, op1=ALU.subtract)
                hs = st.tile([P, nf], DT.float32, tag="hs")    # -cof*scale
                nc.vector.tensor_tensor(hs[:], cof[:], scn[:], op=ALU.mult)

                w = wp.tile([P, tf], DT.uint8, tag="w")
                ot = op.tile([P, tf], DT.float16, tag="o")
                for f in range(nf):
                    s = slice(f * G, (f + 1) * G)
                    rs_f = rs[:, f : f + 1]
                    cof_f = cof[:, f : f + 1]
                    sc_f = sc[:, f : f + 1]
                    hs_f = hs[:, f : f + 1]
                    if f < n_ap + n_aa:
                        nc.scalar.activation(
                            w[:, s], xh[:, s], AF.Identity, bias=cof_f, scale=rs_f)
                        if f < n_ap:
                            nc.gpsimd.tensor_scalar(
                                ot[:, s], w[:, s], sc_f, hs_f, op0=ALU.mult, op1=ALU.add)
                        else:
                            nc.scalar.activation(
                                ot[:, s], w[:, s], AF.Identity, bias=hs_f, scale=sc_f)
                    elif f < n_ap + n_aa + n_pp:
                        nc.gpsimd.tensor_scalar(
                            w[:, s], xh[:, s], rs_f, cof_f, op0=ALU.mult, op1=ALU.add)
                        nc.gpsimd.tensor_scalar(
                            ot[:, s], w[:, s], sc_f, hs_f, op0=ALU.mult, op1=ALU.add)
                    else:
                        nc.vector.tensor_scalar(
                            w[:, s], xh[:, s], rs_f, cof_f, op0=ALU.mult, op1=ALU.add)
                        nc.vector.tensor_scalar(
                            ot[:, s], w[:, s], sc_f, hs_f, op0=ALU.mult, op1=ALU.add)

                pending_out.append((y_d[row0 : row0 + tg, :], ot))
                flush_out(keep=2)

            WF = F // 4
            for s in range(4):
                emit(s * P * WF, WF, ramp=True)
            for t in range(1, N_TILES - 1):
                emit(t * TILE_GROUPS, F)
            for s in range(4):
                emit((N_TILES - 1) * TILE_GROUPS + s * P * WF, WF, ramp=True)
            flush_out(keep=0)

    nc.compile()
    return nc


def _get_compiled():
    global _COMPILED
    if _COMPILED is None:
        _COMPILED = _build()
    return _COMPILED


def kernel(x: np.ndarray) -> np.ndarray:
    assert x.shape == FULL_SHAPE and x.dtype == np.float32, (x.shape, x.dtype)
    nc = _get_compiled()
    flat = np.ascontiguousarray(x).reshape(N_CORES, GROUPS_PER_CORE, G)
    flat16 = flat.astype(np.float16)
    in_maps = [{"x": flat16[i]} for i in range(N_CORES)]
    res = run_bass_kernel_spmd(nc, in_maps, core_ids=list(range(N_CORES)))
    out = np.empty((N_CORES, GROUPS_PER_CORE, G), dtype=np.float32)
    for i in range(N_CORES):
        out[i] = res.results[i]["y"].astype(np.float32)
    return out.reshape(FULL_SHAPE)
